# revision 22
# baseline (speedup 1.0000x reference)
"""Trainium2 Bass kernel for nn_ChannelAttention.

Reference computation (per batch b):
    sourceC = w2 @ context[b]             # [M=4096, L=256], conv1x1
    attn    = softmax(wc[b] @ sourceC)    # [idf=1024, L=256], softmax over L
    attn_c  = attn.T                      # [L, idf]  (output 2)
    out     = (sourceC @ attn_c).T        # [idf, M] -> [idf, ih, iw] (output 1)

Sharding: data-parallel over batch B=16 across 8 cores (2 batches/core),
w2 replicated.  Device kernel per core handles its 2 batches.

Per-batch device plan:
  mm1a (fp32r): sourceC_bf16[m,l]  = (w2T tiles).T @ context        (PE)
  mm1b (fp32r): sourceCT_f32[l,m]  = (context tiles).T @ w2T        (PE)
  wc:   HBM fp32 -> SBUF bf16 via SWDGE cast DMA (natural [i,m]),
        then SBUF->SBUF xbar DMA transpose into wcT bf16 [m,i] tiles.
  mm2 (bf16):  attn_psum[i,l] = sum_m wcT[m,i].T @ sourceC[m,l]     (PE)
  softmax:  rowmax (DVE) -> exp+rowsum (ACT, fused accum) -> recip
            (DVE) -> scale (DVE), all on [i-tile, L] layout.
  attnT: PE transpose of normalized attn -> [l, i] tiles (= attn_c layout)
  mm3 (fp32r): out[i,m] = sum_l attnT[l,i].T @ sourceCT[l,m]        (PE)
"""

import os
from contextlib import ExitStack

import numpy as np

import concourse.bass as bass
import concourse.tile as tile
from concourse import mybir
from concourse.bass_utils import run_bass_kernel_spmd
from concourse.masks import make_identity

N_CORES = 8
B, IDF, M, CDF, L = 16, 1024, 4096, 256, 256
BPC = B // N_CORES  # batches per core

F32 = mybir.dt.float32
F32R = mybir.dt.float32r
BF16 = mybir.dt.bfloat16

P = 128
IT = IDF // P   # 8  i-tiles
MT = M // P     # 32 m-tiles
CT = CDF // P   # 2  c-tiles
LT = L // P     # 2  l-tiles
MCH = M // 512  # 8  m-chunks of 512

# dtype for mm1/mm3 operands.  'bf16' runs at full PE rate; 'f32' is exact
# but 4x slower; 'f32r' is fast fp32 but trips a walrus codegen limit
# ("Too many sync wait commands" on the fp32 self-loading LDWEIGHTS path).
MM13_MODE = os.environ.get("MM13_MODE", "bf16")
DT13 = {"bf16": BF16, "f32": F32, "f32r": F32R}[MM13_MODE]


def build_program():
    """Build the per-core Bass program (identical on all 8 cores)."""
    nc = bass.Bass("TRN2", target_bir_lowering=False, debug=False)

    wc_d = nc.dram_tensor("wc", [BPC, IDF, M], F32, kind="ExternalInput").ap()
    ctx_d = nc.dram_tensor("ctx_in", [BPC, CDF, L], F32, kind="ExternalInput").ap()
    w2t_d = nc.dram_tensor("w2T", [CDF, M], F32, kind="ExternalInput").ap()
    out1_d = nc.dram_tensor("out1", [BPC, IDF, M], F32, kind="ExternalOutput").ap()
    attn_d = nc.dram_tensor("attn_out", [BPC, L, IDF], F32, kind="ExternalOutput").ap()

    with tile.TileContext(nc) as tc:
        with ExitStack() as ctx:
            _tile_kernel(ctx, tc, wc_d, ctx_d, w2t_d, out1_d, attn_d)
    _legalize_dma_waits(nc)
    return nc


def _legalize_dma_waits(nc):
    """Walrus codegen accepts at most ONE sync-wait on queue-dispatched DMA
    instructions (the ISA struct has a single shared wait/update slot) and
    on the matmul/ldweights path.  Tile emits up to 3.  Hoist every wait of
    an offending instruction onto sequencer NOPs inserted immediately
    before it (same engine -> executed in program order by that engine's
    sequencer, so the sync semantics are preserved)."""
    import bass_rust

    Op = nc.isa.Opcode
    one_wait = {
        "InstDMACopy", "InstDmaTransposeAnt", "InstDMAGatherAnt",
        "InstDMAScatterAddAnt", "InstMatmult", "InstLdweights",
    }
    skip = {"InstEventSemaphore", "InstISA",
            "InstUnconditionalBranch", "InstRegisterMove"}
    n_fixed = 0
    for f in nc.m.functions:
        for blk in f.blocks:
            il = blk.instructions
            out = []
            changed = False
            for inst in il:
                si = inst.sync_info
                tn = type(inst).__name__
                limit = 1
                if (si is not None and len(si.on_wait) > limit
                        and tn not in skip):
                    eng = nc.engines[inst.engine]
                    waits = list(si.on_wait)
                    keep, hoist = waits[:limit - 1], waits[limit - 1:]
                    for w in hoist:
                        nop = eng._isa(Op.NEURON_ISA_TPB_OPCODE_NOP, {})
                        nop.sync_info = bass_rust.SyncInfo(
                            on_wait=[w], on_update=[])
                        nc.register_instruction(nop)
                        out.append(nop)
                    inst.sync_info = bass_rust.SyncInfo(
                        on_wait=keep, on_update=list(si.on_update))
                    changed = True
                    n_fixed += 1
                out.append(inst)
            if changed:
                il[:] = out
    return n_fixed


def _tile_kernel(ctx, tc, wc_d, ctx_d, w2t_d, out1_d, attn_d):
    nc = tc.nc

    singles = ctx.enter_context(tc.tile_pool(name="singles", bufs=1))
    sbuf = ctx.enter_context(tc.tile_pool(name="sbuf", bufs=1))
    psum = ctx.enter_context(tc.tile_pool(name="psum", bufs=2, space="PSUM"))

    # Replicated weight: w2T [CDF, M] as CT tiles of [128, M].
    # Casting loads (f32 -> bf16/f32r) require SWDGE (gpsimd).
    ld_eng = nc.sync if DT13 is F32 else nc.gpsimd
    w2t_sb = []
    for ct in range(CT):
        t = singles.tile([P, M], DT13, name=f"w2t_sb{ct}", tag=f"w2t{ct}")
        ld_eng.dma_start(out=t, in_=w2t_d[ct * P:(ct + 1) * P, :])
        w2t_sb.append(t)

    ident = singles.tile([P, P], F32, name="ident", tag="ident")
    make_identity(nc, ident)
    ident_bf = singles.tile([P, P], BF16, name="ident_bf", tag="ident_bf")
    make_identity(nc, ident_bf)

    # PSUM->SBUF copies alternate between DVE and ACT to balance load.
    cp_i = [0]

    def copy(out_ap, in_ap):
        if cp_i[0] % 2 == 0:
            nc.vector.tensor_copy(out_ap, in_ap)
        else:
            nc.scalar.copy(out_ap, in_ap)
        cp_i[0] += 1

    for b in range(BPC):
        # ---- load context tiles [c, l] ----
        ctx_sb = []
        for ct in range(CT):
            t = sbuf.tile([P, L], DT13, name=f"ctx{ct}_{b}", tag=f"ctx{ct}", bufs=2)
            ld_eng.dma_start(out=t, in_=ctx_d[b, ct * P:(ct + 1) * P, :])
            ctx_sb.append(t)

        # ---- mm1a: sourceC bf16 [m-tile][128, L] ----
        src_c = []
        for mt in range(MT):
            ps = psum.tile([P, L], F32, name=f"ps1a_{b}_{mt}", tag="psB", bufs=2)
            for ct in range(CT):
                nc.tensor.matmul(
                    ps,
                    w2t_sb[ct][:, mt * P:(mt + 1) * P],
                    ctx_sb[ct],
                    start=(ct == 0),
                    stop=(ct == CT - 1),
                )
            t = sbuf.tile([P, L], BF16, name=f"srcC{mt}_{b}", tag=f"srcC{mt}")
            copy(t, ps)
            src_c.append(t)

        # ---- mm1b: sourceCT [l-tile][128, M] (f32r for mm3) ----
        src_ct = []
        for lt in range(LT):
            t = sbuf.tile([P, M], DT13, name=f"srcCT{lt}_{b}", tag=f"srcCT{lt}")
            src_ct.append(t)
            for mc in range(MCH):
                ps = psum.tile([P, 512], F32, name=f"ps1b_{b}_{lt}_{mc}",
                               tag="psA", bufs=2)
                for ct in range(CT):
                    nc.tensor.matmul(
                        ps,
                        ctx_sb[ct][:, lt * P:(lt + 1) * P],
                        w2t_sb[ct][:, mc * 512:(mc + 1) * 512],
                        start=(ct == 0),
                        stop=(ct == CT - 1),
                    )
                copy(t[:, mc * 512:(mc + 1) * 512], ps)

        # ---- wc: load (fp32->bf16 cast DMA) + PE transpose to wcT ----
        wct = []
        for mt in range(MT):
            t = sbuf.tile([P, IDF], BF16, name=f"wcT{mt}_{b}", tag=f"wcT{mt}")
            wct.append(t)
        for it in range(IT):
            for mc in range(MCH):
                nat = sbuf.tile([P, 512], BF16, name=f"nat_{b}_{it}_{mc}",
                                tag="wcnat", bufs=6)
                nc.gpsimd.dma_start(
                    out=nat,
                    in_=wc_d[b, it * P:(it + 1) * P, mc * 512:(mc + 1) * 512],
                )
                for k in range(4):
                    mt = mc * 4 + k
                    tps = psum.tile([P, P], BF16, name=f"tps_{b}_{it}_{mt}",
                                    tag="psW", bufs=2)
                    nc.tensor.transpose(tps, nat[:, k * P:(k + 1) * P], ident_bf)
                    copy(wct[mt][:, it * P:(it + 1) * P], tps)

        # ---- mm2 (bf16) + softmax + attnT ----
        attn_t = []
        attn_t32 = []
        for lt in range(LT):
            t = sbuf.tile([P, IDF], DT13, name=f"attnT{lt}_{b}", tag=f"attnT{lt}")
            attn_t.append(t)
            t32 = sbuf.tile([P, IDF], F32, name=f"attnT32_{lt}_{b}",
                            tag=f"attnT32_{lt}")
            attn_t32.append(t32)
        for it in range(IT):
            aps = psum.tile([P, L], F32, name=f"ps2_{b}_{it}", tag="psB", bufs=2)
            for mt in range(MT):
                nc.tensor.matmul(
                    aps,
                    wct[mt][:, it * P:(it + 1) * P],
                    src_c[mt],
                    start=(mt == 0),
                    stop=(mt == MT - 1),
                )
            neg_mx = sbuf.tile([P, 1], F32, name=f"nmx_{b}_{it}", tag="nmx", bufs=2)
            nc.vector.reduce_max(neg_mx, aps, axis=mybir.AxisListType.X, negate=True)
            e = sbuf.tile([P, L], F32, name=f"e_{b}_{it}", tag="e", bufs=2)
            s = sbuf.tile([P, 1], F32, name=f"s_{b}_{it}", tag="s", bufs=2)
            nc.scalar.activation(e, aps, mybir.ActivationFunctionType.Exp,
                                 bias=neg_mx, scale=1.0, accum_out=s)
            r = sbuf.tile([P, 1], F32, name=f"r_{b}_{it}", tag="r", bufs=2)
            nc.vector.reciprocal(r, s)
            a = sbuf.tile([P, L], F32, name=f"a_{b}_{it}", tag="a", bufs=2)
            nc.vector.tensor_scalar_mul(a, e, r)
            for lt in range(LT):
                tp = psum.tile([P, P], F32, name=f"pstr_{b}_{it}_{lt}",
                               tag="psT", bufs=2)
                nc.tensor.transpose(tp, a[:, lt * P:(lt + 1) * P], ident)
                copy(attn_t[lt][:, it * P:(it + 1) * P], tp)
                # full-precision copy feeding the attn_c output
                copy(attn_t32[lt][:, it * P:(it + 1) * P], tp)
        for lt in range(LT):
            nc.sync.dma_start(out=attn_d[b, lt * P:(lt + 1) * P, :],
                              in_=attn_t32[lt])

        # ---- mm3 (fp32r): out1[b][i, m] ----
        for it in range(IT):
            for mc in range(MCH):
                ps = psum.tile([P, 512], F32, name=f"ps3_{b}_{it}_{mc}",
                               tag="psA", bufs=2)
                for lt in range(LT):
                    nc.tensor.matmul(
                        ps,
                        attn_t[lt][:, it * P:(it + 1) * P],
                        src_ct[lt][:, mc * 512:(mc + 1) * 512],
                        start=(lt == 0),
                        stop=(lt == LT - 1),
                    )
                o = sbuf.tile([P, 512], F32, name=f"o_{b}_{it}_{mc}",
                              tag="o", bufs=4)
                copy(o, ps)
                nc.sync.dma_start(
                    out=out1_d[b, it * P:(it + 1) * P, mc * 512:(mc + 1) * 512],
                    in_=o,
                )


_CACHE = {}


def _get_nc():
    if "nc" not in _CACHE:
        _CACHE["nc"] = build_program()
    return _CACHE["nc"]


def make_in_maps(weightedContext, context, w2):
    wc = np.ascontiguousarray(np.asarray(weightedContext, dtype=np.float32))
    cx = np.ascontiguousarray(np.asarray(context, dtype=np.float32))
    w2t = np.ascontiguousarray(np.asarray(w2, dtype=np.float32).T)
    in_maps = []
    for c in range(N_CORES):
        sl = slice(c * BPC, (c + 1) * BPC)
        in_maps.append({
            "wc": np.ascontiguousarray(wc[sl]),
            "ctx_in": np.ascontiguousarray(cx[sl]),
            "w2T": w2t,
        })
    return in_maps


def run_spmd(in_maps, trace=False, **kw):
    nc = _get_nc()
    return run_bass_kernel_spmd(nc, in_maps, list(range(N_CORES)), trace=trace, **kw)


def kernel(weightedContext, context, w2, ih, iw):
    ih, iw = int(ih), int(iw)
    res = run_spmd(make_in_maps(weightedContext, context, w2))
    outs = res.results
    out1 = np.concatenate([o["out1"] for o in outs], axis=0)
    attn_c = np.concatenate([o["attn_out"] for o in outs], axis=0)
    out1 = np.ascontiguousarray(out1).reshape(B, IDF, ih, iw).astype(np.float32)
    attn_c = np.ascontiguousarray(attn_c).astype(np.float32)
    return out1, attn_c


# revision 23
# speedup vs baseline: 1.0151x; 1.0151x over previous
"""Trainium2 Bass kernel for nn_ChannelAttention.

Reference computation (per batch b):
    sourceC = w2 @ context[b]             # [M=4096, L=256], conv1x1
    attn    = softmax(wc[b] @ sourceC)    # [idf=1024, L=256], softmax over L
    attn_c  = attn.T                      # [L, idf]  (output 2)
    out     = (sourceC @ attn_c).T        # [idf, M] -> [idf, ih, iw] (output 1)

Sharding: data-parallel over batch B=16 across 8 cores (2 batches/core),
w2 replicated.  Device kernel per core handles its 2 batches.

Per-batch device plan:
  mm1a (fp32r): sourceC_bf16[m,l]  = (w2T tiles).T @ context        (PE)
  mm1b (fp32r): sourceCT_f32[l,m]  = (context tiles).T @ w2T        (PE)
  wc:   HBM fp32 -> SBUF bf16 via SWDGE cast DMA (natural [i,m]),
        then SBUF->SBUF xbar DMA transpose into wcT bf16 [m,i] tiles.
  mm2 (bf16):  attn_psum[i,l] = sum_m wcT[m,i].T @ sourceC[m,l]     (PE)
  softmax:  rowmax (DVE) -> exp+rowsum (ACT, fused accum) -> recip
            (DVE) -> scale (DVE), all on [i-tile, L] layout.
  attnT: PE transpose of normalized attn -> [l, i] tiles (= attn_c layout)
  mm3 (fp32r): out[i,m] = sum_l attnT[l,i].T @ sourceCT[l,m]        (PE)
"""

import os
from contextlib import ExitStack

import numpy as np

import concourse.bass as bass
import concourse.tile as tile
from concourse import mybir
from concourse.bass_utils import run_bass_kernel_spmd
from concourse.masks import make_identity

N_CORES = 8
B, IDF, M, CDF, L = 16, 1024, 4096, 256, 256
BPC = B // N_CORES  # batches per core

F32 = mybir.dt.float32
F32R = mybir.dt.float32r
BF16 = mybir.dt.bfloat16

P = 128
IT = IDF // P   # 8  i-tiles
MT = M // P     # 32 m-tiles
CT = CDF // P   # 2  c-tiles
LT = L // P     # 2  l-tiles
MCH = M // 512  # 8  m-chunks of 512

# dtype for mm1/mm3 operands.  'bf16' runs at full PE rate; 'f32' is exact
# but 4x slower; 'f32r' is fast fp32 but trips a walrus codegen limit
# ("Too many sync wait commands" on the fp32 self-loading LDWEIGHTS path).
MM13_MODE = os.environ.get("MM13_MODE", "bf16")
DT13 = {"bf16": BF16, "f32": F32, "f32r": F32R}[MM13_MODE]


def build_program():
    """Build the per-core Bass program (identical on all 8 cores)."""
    nc = bass.Bass("TRN2", target_bir_lowering=False, debug=False)

    wc_d = nc.dram_tensor("wc", [BPC, IDF, M], F32, kind="ExternalInput").ap()
    ctx_d = nc.dram_tensor("ctx_in", [BPC, CDF, L], F32, kind="ExternalInput").ap()
    w2t_d = nc.dram_tensor("w2T", [CDF, M], F32, kind="ExternalInput").ap()
    out1_d = nc.dram_tensor("out1", [BPC, IDF, M], F32, kind="ExternalOutput").ap()
    attn_d = nc.dram_tensor("attn_out", [BPC, L, IDF], F32, kind="ExternalOutput").ap()

    with tile.TileContext(nc) as tc:
        with ExitStack() as ctx:
            _tile_kernel(ctx, tc, wc_d, ctx_d, w2t_d, out1_d, attn_d)
    _legalize_dma_waits(nc)
    return nc


def _legalize_dma_waits(nc):
    """Walrus codegen accepts at most ONE sync-wait on queue-dispatched DMA
    instructions (the ISA struct has a single shared wait/update slot) and
    on the matmul/ldweights path.  Tile emits up to 3.  Hoist every wait of
    an offending instruction onto sequencer NOPs inserted immediately
    before it (same engine -> executed in program order by that engine's
    sequencer, so the sync semantics are preserved)."""
    import bass_rust

    Op = nc.isa.Opcode
    one_wait = {
        "InstDMACopy", "InstDmaTransposeAnt", "InstDMAGatherAnt",
        "InstDMAScatterAddAnt", "InstMatmult", "InstLdweights",
    }
    skip = {"InstEventSemaphore", "InstISA",
            "InstUnconditionalBranch", "InstRegisterMove"}
    n_fixed = 0
    for f in nc.m.functions:
        for blk in f.blocks:
            il = blk.instructions
            out = []
            changed = False
            for inst in il:
                si = inst.sync_info
                tn = type(inst).__name__
                limit = 1
                if (si is not None and len(si.on_wait) > limit
                        and tn not in skip):
                    eng = nc.engines[inst.engine]
                    waits = list(si.on_wait)
                    keep, hoist = waits[:limit - 1], waits[limit - 1:]
                    for w in hoist:
                        nop = eng._isa(Op.NEURON_ISA_TPB_OPCODE_NOP, {})
                        nop.sync_info = bass_rust.SyncInfo(
                            on_wait=[w], on_update=[])
                        nc.register_instruction(nop)
                        out.append(nop)
                    inst.sync_info = bass_rust.SyncInfo(
                        on_wait=keep, on_update=list(si.on_update))
                    changed = True
                    n_fixed += 1
                out.append(inst)
            if changed:
                il[:] = out
    return n_fixed


def _tile_kernel(ctx, tc, wc_d, ctx_d, w2t_d, out1_d, attn_d):
    nc = tc.nc

    singles = ctx.enter_context(tc.tile_pool(name="singles", bufs=1))
    sbuf = ctx.enter_context(tc.tile_pool(name="sbuf", bufs=1))
    psum = ctx.enter_context(tc.tile_pool(name="psum", bufs=2, space="PSUM"))

    # Replicated weight: w2T [CDF, M] as CT tiles of [128, M].
    # Casting loads (f32 -> bf16/f32r) require SWDGE (gpsimd).
    ld_eng = nc.sync if DT13 is F32 else nc.gpsimd
    w2t_sb = []
    for ct in range(CT):
        t = singles.tile([P, M], DT13, name=f"w2t_sb{ct}", tag=f"w2t{ct}")
        ld_eng.dma_start(out=t, in_=w2t_d[ct * P:(ct + 1) * P, :])
        w2t_sb.append(t)

    ident = singles.tile([P, P], F32, name="ident", tag="ident")
    make_identity(nc, ident)
    ident_bf = singles.tile([P, P], BF16, name="ident_bf", tag="ident_bf")
    make_identity(nc, ident_bf)

    # PSUM->SBUF copies alternate between DVE and ACT to balance load.
    cp_i = [0]

    def copy(out_ap, in_ap):
        if cp_i[0] % 2 == 0:
            nc.vector.tensor_copy(out_ap, in_ap)
        else:
            nc.scalar.copy(out_ap, in_ap)
        cp_i[0] += 1

    for b in range(BPC):
        # ---- load context tiles [c, l] ----
        ctx_sb = []
        for ct in range(CT):
            t = sbuf.tile([P, L], DT13, name=f"ctx{ct}_{b}", tag=f"ctx{ct}", bufs=2)
            ld_eng.dma_start(out=t, in_=ctx_d[b, ct * P:(ct + 1) * P, :])
            ctx_sb.append(t)

        # ---- mm1a: sourceC bf16 [m-tile][128, L] ----
        src_c = []
        for mt in range(MT):
            ps = psum.tile([P, L], F32, name=f"ps1a_{b}_{mt}", tag="psB", bufs=2)
            for ct in range(CT):
                nc.tensor.matmul(
                    ps,
                    w2t_sb[ct][:, mt * P:(mt + 1) * P],
                    ctx_sb[ct],
                    start=(ct == 0),
                    stop=(ct == CT - 1),
                )
            t = sbuf.tile([P, L], BF16, name=f"srcC{mt}_{b}", tag=f"srcC{mt}")
            copy(t, ps)
            src_c.append(t)

        # ---- mm1b: sourceCT [l-tile][128, M] (f32r for mm3) ----
        src_ct = []
        for lt in range(LT):
            t = sbuf.tile([P, M], DT13, name=f"srcCT{lt}_{b}", tag=f"srcCT{lt}")
            src_ct.append(t)
            for mc in range(MCH):
                ps = psum.tile([P, 512], F32, name=f"ps1b_{b}_{lt}_{mc}",
                               tag="psA", bufs=2)
                for ct in range(CT):
                    nc.tensor.matmul(
                        ps,
                        ctx_sb[ct][:, lt * P:(lt + 1) * P],
                        w2t_sb[ct][:, mc * 512:(mc + 1) * 512],
                        start=(ct == 0),
                        stop=(ct == CT - 1),
                    )
                copy(t[:, mc * 512:(mc + 1) * 512], ps)

        # ---- wc: load (fp32->bf16 cast DMA) + PE transpose to wcT ----
        wct = []
        for mt in range(MT):
            t = sbuf.tile([P, IDF], BF16, name=f"wcT{mt}_{b}", tag=f"wcT{mt}")
            wct.append(t)
        for it in range(IT):
            for mc in range(MCH):
                nat = sbuf.tile([P, 512], BF16, name=f"nat_{b}_{it}_{mc}",
                                tag="wcnat", bufs=10)
                nc.gpsimd.dma_start(
                    out=nat,
                    in_=wc_d[b, it * P:(it + 1) * P, mc * 512:(mc + 1) * 512],
                )
                for k in range(4):
                    mt = mc * 4 + k
                    tps = psum.tile([P, P], BF16, name=f"tps_{b}_{it}_{mt}",
                                    tag="psW", bufs=2)
                    nc.tensor.transpose(tps, nat[:, k * P:(k + 1) * P], ident_bf)
                    copy(wct[mt][:, it * P:(it + 1) * P], tps)

        # ---- mm2 (bf16) + softmax + attnT ----
        attn_t = []
        attn_t32 = []
        for lt in range(LT):
            t = sbuf.tile([P, IDF], DT13, name=f"attnT{lt}_{b}", tag=f"attnT{lt}")
            attn_t.append(t)
            t32 = sbuf.tile([P, IDF], F32, name=f"attnT32_{lt}_{b}",
                            tag=f"attnT32_{lt}")
            attn_t32.append(t32)
        for it in range(IT):
            aps = psum.tile([P, L], F32, name=f"ps2_{b}_{it}", tag="psB", bufs=2)
            for mt in range(MT):
                nc.tensor.matmul(
                    aps,
                    wct[mt][:, it * P:(it + 1) * P],
                    src_c[mt],
                    start=(mt == 0),
                    stop=(mt == MT - 1),
                )
            neg_mx = sbuf.tile([P, 1], F32, name=f"nmx_{b}_{it}", tag="nmx", bufs=2)
            nc.vector.reduce_max(neg_mx, aps, axis=mybir.AxisListType.X, negate=True)
            e = sbuf.tile([P, L], F32, name=f"e_{b}_{it}", tag="e", bufs=2)
            s = sbuf.tile([P, 1], F32, name=f"s_{b}_{it}", tag="s", bufs=2)
            nc.scalar.activation(e, aps, mybir.ActivationFunctionType.Exp,
                                 bias=neg_mx, scale=1.0, accum_out=s)
            r = sbuf.tile([P, 1], F32, name=f"r_{b}_{it}", tag="r", bufs=2)
            nc.vector.reciprocal(r, s)
            a = sbuf.tile([P, L], F32, name=f"a_{b}_{it}", tag="a", bufs=2)
            nc.vector.tensor_scalar_mul(a, e, r)
            for lt in range(LT):
                tp = psum.tile([P, P], F32, name=f"pstr_{b}_{it}_{lt}",
                               tag="psT", bufs=2)
                nc.tensor.transpose(tp, a[:, lt * P:(lt + 1) * P], ident)
                copy(attn_t[lt][:, it * P:(it + 1) * P], tp)
                # full-precision copy feeding the attn_c output
                copy(attn_t32[lt][:, it * P:(it + 1) * P], tp)
        for lt in range(LT):
            nc.sync.dma_start(out=attn_d[b, lt * P:(lt + 1) * P, :],
                              in_=attn_t32[lt])

        # ---- mm3 (fp32r): out1[b][i, m] ----
        for it in range(IT):
            for mc in range(MCH):
                ps = psum.tile([P, 512], F32, name=f"ps3_{b}_{it}_{mc}",
                               tag="psA", bufs=2)
                for lt in range(LT):
                    nc.tensor.matmul(
                        ps,
                        attn_t[lt][:, it * P:(it + 1) * P],
                        src_ct[lt][:, mc * 512:(mc + 1) * 512],
                        start=(lt == 0),
                        stop=(lt == LT - 1),
                    )
                o = sbuf.tile([P, 512], F32, name=f"o_{b}_{it}_{mc}",
                              tag="o", bufs=6)
                copy(o, ps)
                nc.sync.dma_start(
                    out=out1_d[b, it * P:(it + 1) * P, mc * 512:(mc + 1) * 512],
                    in_=o,
                )


_CACHE = {}


def _get_nc():
    if "nc" not in _CACHE:
        _CACHE["nc"] = build_program()
    return _CACHE["nc"]


def make_in_maps(weightedContext, context, w2):
    wc = np.ascontiguousarray(np.asarray(weightedContext, dtype=np.float32))
    cx = np.ascontiguousarray(np.asarray(context, dtype=np.float32))
    w2t = np.ascontiguousarray(np.asarray(w2, dtype=np.float32).T)
    in_maps = []
    for c in range(N_CORES):
        sl = slice(c * BPC, (c + 1) * BPC)
        in_maps.append({
            "wc": np.ascontiguousarray(wc[sl]),
            "ctx_in": np.ascontiguousarray(cx[sl]),
            "w2T": w2t,
        })
    return in_maps


def run_spmd(in_maps, trace=False, **kw):
    nc = _get_nc()
    return run_bass_kernel_spmd(nc, in_maps, list(range(N_CORES)), trace=trace, **kw)


def kernel(weightedContext, context, w2, ih, iw):
    ih, iw = int(ih), int(iw)
    res = run_spmd(make_in_maps(weightedContext, context, w2))
    outs = res.results
    out1 = np.concatenate([o["out1"] for o in outs], axis=0)
    attn_c = np.concatenate([o["attn_out"] for o in outs], axis=0)
    out1 = np.ascontiguousarray(out1).reshape(B, IDF, ih, iw).astype(np.float32)
    attn_c = np.ascontiguousarray(attn_c).astype(np.float32)
    return out1, attn_c
